# revision 29
# baseline (speedup 1.0000x reference)
"""Trainium2 Bass kernel for nn_AttentionModel_47648367182457.

Pointer-network attention (Kool et al. style): one query per batch attends over
N=1024 node embeddings. Instead of materializing the O(B*N*D*3D) projection
kvl = E @ W_node (103 GFLOP), the query/glimpse vectors are back-projected
through the weights so every pass over E is a thin matmul:

  compat[b,h,n] = E[b,n,:] . qt[b,h,:]   qt = W_gk-block @ query    (contract d)
  wsum[b,h,:]  = sum_n attn[b,h,n] E[b,n,:]                         (contract n)
  logits[b,n]  = E[b,n,:] . gt[b,:]      gt = W_lk^T @ glimpse      (contract d)

Total ~2.3 GFLOP + one read of E (256 MB). Data-parallel over batch: 8 cores x
32 batches. E is transposed on-chip (PE transpose) because the d-contractions
need d on partitions; all matmuls run in float32r (~1.5e-4 rel err on HW at
1 cyc/col streaming).
"""
import sys

sys.path.insert(0, "/opt/trn_rl_repo")

import numpy as np

import concourse.bass as bass
import concourse.bacc as bacc
import concourse.tile as tile
from concourse import mybir
from concourse import bass_utils

F32 = mybir.dt.float32
F32R = mybir.dt.float32r
BF16 = mybir.dt.bfloat16

B, N, D, H = 256, 1024, 256, 8
DK = D // H              # 32
NCORES = 8
BPC = B // NCORES        # 32 batches per core
NCH = N // 128           # 8 n-chunks of 128
TANH_CLIP = 10.0
INV_SQRT_DK = 1.0 / np.sqrt(DK)
INV_SQRT_D = 1.0 / np.sqrt(D)

_CACHE = {}


def _view(ap, free_dims):
    """AP with same tensor/partition dim but custom free [step, count] list."""
    return bass.AP(tensor=ap.tensor, offset=ap.offset,
                   ap=[ap.ap[0]] + [list(e) for e in free_dims])


def _bcast_free(ap, n_rep):
    return bass.AP(tensor=ap.tensor, offset=ap.offset,
                   ap=list(ap.ap) + [[0, n_rep]])


def build_program():
    nc = bacc.Bacc("TRN2", target_bir_lowering=False, debug=False,
                   num_devices=NCORES)

    emb = nc.dram_tensor("emb", [BPC, N, D], BF16, kind="ExternalInput").ap()
    kmT_d = nc.dram_tensor("kmT", [128, NCH, BPC], BF16, kind="ExternalInput").ap()
    mT_d = nc.dram_tensor("mT", [128, NCH, BPC], mybir.dt.uint8, kind="ExternalInput").ap()
    scT_d = nc.dram_tensor("scT", [128, 5, BPC], F32R, kind="ExternalInput").ap()
    wgvb_d = nc.dram_tensor("wgvb", [128, 2, D], BF16, kind="ExternalInput").ap()
    wnT_d = nc.dram_tensor("wnT", [128, 6, D], F32R, kind="ExternalInput").ap()
    wfx_d = nc.dram_tensor("wfx", [128, 2, D], F32R, kind="ExternalInput").ap()
    wsp_d = nc.dram_tensor("wsp", [128, 5, D], F32R, kind="ExternalInput").ap()
    wout_d = nc.dram_tensor("wout", [128, 2, D], F32R, kind="ExternalInput").ap()
    hm_d = nc.dram_tensor("hm", [128, 2, H], F32, kind="ExternalInput").ap()
    ident_d = nc.dram_tensor("ident", [128, 128], F32R, kind="ExternalInput").ap()
    identf_d = nc.dram_tensor("identf", [128, 128], F32, kind="ExternalInput").ap()
    out_d = nc.dram_tensor("out", [BPC, N], F32, kind="ExternalOutput").ap()

    with tile.TileContext(nc) as tc:
        with tc.tile_pool(name="wpool", bufs=1) as wpool, \
             tc.tile_pool(name="epool", bufs=10) as epool, \
             tc.tile_pool(name="etpool", bufs=10) as etpool, \
             tc.tile_pool(name="spool", bufs=9) as spool, \
             tc.tile_pool(name="fpool", bufs=1) as fpool, \
             tc.tile_pool(name="ps_wp", bufs=1, space="PSUM") as ps_wp, \
             tc.tile_pool(name="ps_sm", bufs=7, space="PSUM") as ps_sm:

            # ------------- setup: constants / weights (DMA-cast to f32r) ----
            def load_r(name, shape, dram_ap):
                t = wpool.tile(shape, F32R, tag=name)
                nc.sync.dma_start(out=t, in_=dram_ap)
                return t

            ident = load_r("ident", [128, 128], ident_d)
            wgvb = wpool.tile([128, 2, D], BF16, tag="wgvb")
            nc.sync.dma_start(out=wgvb, in_=wgvb_d)
            wnT = load_r("wnT", [128, 6, D], wnT_d)
            wfx = load_r("wfx", [128, 2, D], wfx_d)
            wsp = load_r("wsp", [128, 5, D], wsp_d)
            wout = load_r("wout", [128, 2, D], wout_d)
            scT = load_r("scT", [128, 5, BPC], scT_d)

            ident_f = wpool.tile([128, 128], F32, tag="ident_f")
            nc.sync.dma_start(out=ident_f, in_=identf_d)
            hm = wpool.tile([128, 2, H], F32, tag="hm")
            nc.sync.dma_start(out=hm, in_=hm_d)

            ones_f = wpool.tile([128, 1], F32, tag="ones_f")
            nc.vector.memset(ones_f, 1.0)
            ones = wpool.tile([128, 1], F32R, tag="ones")
            nc.vector.tensor_copy(out=ones, in_=ones_f)
            onesrow_f = wpool.tile([1, 128], F32, tag="onesrow_f")
            nc.vector.memset(onesrow_f, 1.0)
            ones_row = wpool.tile([1, 128], F32R, tag="ones_row")
            nc.vector.tensor_copy(out=ones_row, in_=onesrow_f)
            ninf = wpool.tile([128, 1], F32, tag="ninf")
            nc.vector.memset(ninf, float("-inf"))
            zeros = wpool.tile([128, 1], F32, tag="zeros")
            nc.vector.memset(zeros, 0.0)
            ones_b = wpool.tile([128, 1], BF16, tag="ones_b")
            nc.vector.memset(ones_b, 1.0)

            # kmT[p, c, b]: keep-multiplier with n on partitions (f32, DVE use)
            kmT = wpool.tile([128, NCH, BPC], BF16, tag="kmT")
            nc.sync.dma_start(out=kmT, in_=kmT_d)
            maskT = wpool.tile([128, NCH, BPC], mybir.dt.uint8, tag="maskT")
            nc.sync.dma_start(out=maskT, in_=mT_d)

            # sc-tilde^T: scq[j(part), jh, b]
            scq = wpool.tile([128, 2, BPC], F32R, tag="scq")
            for jh in range(2):
                sp = ps_sm.tile([128, BPC], F32, tag="sm")
                for jj in range(5):
                    nc.tensor.matmul(
                        sp, wsp[:, jj, jh * 128:(jh + 1) * 128],
                        scT[:, jj, :], start=(jj == 0), stop=(jj == 4))
                nc.vector.tensor_copy(out=scq[:, jh, :], in_=sp)

            # ---------------- per-batch pipeline (software-pipelined) ----
            raw_all = fpool.tile([128, BPC, NCH], F32, tag="raw_all")
            state = {}

            def stageA(b):
                st = {}
                Enat = epool.tile([128, NCH, D], BF16, tag="enat")
                nc.sync.dma_start(
                    out=Enat,
                    in_=emb[b].rearrange("(c p) d -> p c d", p=128))
                ET = etpool.tile([128, 2, N], BF16, tag="et")
                for dh in range(2):
                    nc.sync.dma_start(
                        out=ET[:, dh, :],
                        in_=emb[b][:, dh * 128:(dh + 1) * 128],
                        transpose=True)
                mp = ps_sm.tile([1, D], F32, tag="sm")
                for c in range(NCH):
                    nc.tensor.matmul(
                        mp, ones_b, Enat[:, c, :],
                        start=(c == 0), stop=(c == NCH - 1))
                meanS = spool.tile([1, D], F32, tag="meanS")
                nc.scalar.copy(out=meanS, in_=mp)
                mtp = ps_sm.tile([128, 2], F32, tag="sm")
                for eh in range(2):
                    nc.tensor.transpose(
                        mtp[:, eh:eh + 1],
                        meanS[:, eh * 128:(eh + 1) * 128],
                        ident_f[0:1, 0:1])
                meanET = spool.tile([128, 2], BF16, tag="meanET")
                nc.vector.tensor_copy(out=meanET, in_=mtp)
                st["Enat"], st["ET"], st["meanET"] = Enat, ET, meanET
                state[b] = st

            def stageB(b):
                st = state[b]
                Enat, ET, meanET = st["Enat"], st["ET"], st["meanET"]
                qtp = ps_sm.tile([128, 2], F32, tag="sm")
                for jh in range(2):
                    for eh in range(2):
                        nc.tensor.matmul(
                            qtp[:, jh:jh + 1],
                            wfxb[:, eh, jh * 128:(jh + 1) * 128],
                            meanET[:, eh:eh + 1],
                            start=(eh == 0), stop=(eh == 1))
                qTs = spool.tile([128, 2], F32, tag="qTs")
                nc.vector.tensor_add(out=qTs, in0=qtp, in1=scq[:, :, b])
                st["qTs"] = qTs

            def stageB1b(b):
                st = state[b]
                qTs = st["qTs"]
                Qm = spool.tile([128, 2, H], F32R, tag="Qm")
                nc.gpsimd.tensor_mul(out=Qm, in0=hm, in1=_bcast_free(qTs, H))
                qp = ps_sm.tile([128, 2, H], F32, tag="sm")
                for dh in range(2):
                    for jh in range(2):
                        nc.tensor.matmul(
                            qp[:, dh, :],
                            wnT[:, jh, dh * 128:(dh + 1) * 128],
                            Qm[:, jh, :],
                            start=(jh == 0), stop=(jh == 1))
                qtS = spool.tile([128, 2, H], BF16, tag="qtS")
                nc.vector.tensor_copy(out=qtS, in_=qp)
                st["qtS"] = qtS

            def stageB2(b):
                st = state[b]
                ET, qtS = st["ET"], st["qtS"]
                ctp = ps_sm.tile([128, NCH * H], F32, tag="sm")
                for c in range(NCH):
                    for dh in range(2):
                        nc.tensor.matmul(
                            ctp[:, c * H:(c + 1) * H],
                            ET[:, dh, c * 128:(c + 1) * 128],
                            qtS[:, dh, :],
                            start=(dh == 0), stop=(dh == 1))
                expT = spool.tile([128, NCH, H], BF16, tag="expT")
                nc.scalar.activation(
                    out=expT.rearrange("p c h -> p (c h)"), in_=ctp,
                    func=mybir.ActivationFunctionType.Exp)
                nc.gpsimd.tensor_mul(
                    out=expT, in0=expT, in1=_bcast_free(kmT[:, :, b], H))
                st["expT"] = expT

            def stageC(b):
                st = state[b]
                Enat, expT = st["Enat"], st["expT"]
                part = spool.tile([128, H], F32, tag="part")
                nc.vector.reduce_sum(
                    out=part,
                    in_=_view(expT, [[1, H], [H, NCH]]),
                    axis=mybir.AxisListType.X)
                dp = ps_sm.tile([H, 1], F32, tag="sm")
                nc.tensor.matmul(dp, part, ones_f, start=True, stop=True)
                rd = spool.tile([H, 1], F32, tag="rd")
                nc.vector.reciprocal(out=rd, in_=dp)
                wp = ps_wp.tile([H, D], F32, tag="wp")
                for c in range(NCH):
                    nc.tensor.matmul(
                        wp, expT[:, c, :], Enat[:, c, :],
                        start=(c == 0), stop=(c == NCH - 1))
                wsumS = spool.tile([H, D], BF16, tag="wsumS")
                nc.scalar.activation(
                    out=wsumS, in_=wp,
                    func=mybir.ActivationFunctionType.Copy, scale=rd)
                wtp = ps_sm.tile([128, 2 * H], BF16, tag="sm")
                for dh in range(2):
                    nc.tensor.transpose(
                        wtp[:, dh * H:(dh + 1) * H],
                        wsumS[:, dh * 128:(dh + 1) * 128],
                        ident_b[0:H, 0:H])
                wsumT = spool.tile([128, 2, H], BF16, tag="wsumT")
                nc.vector.tensor_copy(out=wsumT, in_=wtp)
                st["wsumT"] = wsumT

            def stageC2(b):
                st = state[b]
                wsumT = st["wsumT"]
                htp = ps_sm.tile([128, 2], F32, tag="sm")
                for h in range(H):
                    r0 = 32 * (h % 4)
                    col = h // 4
                    for dh in range(2):
                        nc.tensor.matmul(
                            htp[r0:r0 + 32, col:col + 1],
                            wgvb[:, dh, 32 * h: 32 * h + 32],
                            wsumT[:, dh, h:h + 1],
                            start=(dh == 0), stop=(dh == 1),
                            tile_position=(0, r0))
                htS = spool.tile([128, 2], BF16, tag="htS")
                nc.vector.tensor_copy(out=htS, in_=htp)
                st["htS"] = htS

            def stageC2b(b):
                st = state[b]
                htS = st["htS"]
                glp = ps_sm.tile([128, 2], F32, tag="sm")
                for dph in range(2):
                    for hh in range(2):
                        nc.tensor.matmul(
                            glp[:, dph:dph + 1],
                            woutb[:, hh, dph * 128:(dph + 1) * 128],
                            htS[:, hh:hh + 1],
                            start=(hh == 0), stop=(hh == 1))
                glT = spool.tile([128, 2], BF16, tag="glT")
                nc.vector.tensor_copy(out=glT, in_=glp)
                g2tp = ps_sm.tile([128, 2], F32, tag="sm")
                for dh in range(2):
                    for dph in range(2):
                        nc.tensor.matmul(
                            g2tp[:, dh:dh + 1],
                            wlkb[:, dph, dh * 128:(dh + 1) * 128],
                            glT[:, dph:dph + 1],
                            start=(dph == 0), stop=(dph == 1))
                gtT = spool.tile([128, 2], BF16, tag="gtT")
                nc.vector.tensor_scalar_mul(
                    out=gtT, in0=g2tp, scalar1=float(INV_SQRT_D))
                st["gtT"] = gtT

            def stageD(b):
                st = state.pop(b)
                ET, gtT = st["ET"], st["gtT"]
                ltp = ps_sm.tile([128, 2 * NCH], F32, tag="sm")
                for c in range(NCH):
                    for dh in range(2):
                        nc.tensor.matmul(
                            ltp[:, 2 * c:2 * c + 2],
                            ET[:, dh, c * 128:(c + 1) * 128],
                            _bcast_free(gtT[:, dh:dh + 1], 2),
                            start=(dh == 0), stop=(dh == 1))
                nc.vector.tensor_copy(out=raw_all[:, b, :],
                                      in_=_view(ltp, [[2, NCH]]))

            stages = [stageA, stageB, stageB1b, stageB2, stageC,
                      stageC2, stageC2b, stageD]
            NST = len(stages)
            for step in range(BPC + NST - 1):
                for si, fn in enumerate(stages):
                    bb = step - si
                    if 0 <= bb < BPC:
                        fn(bb)

            # ---------------- batched finale over all 32 b ----------------
            flat = raw_all.rearrange("p b c -> p (b c)")
            l10 = fpool.tile([128, BPC, NCH], F32, tag="l10")
            l10f = l10.rearrange("p b c -> p (b c)")
            nc.scalar.activation(out=l10f, in_=flat,
                                 func=mybir.ActivationFunctionType.Tanh)
            nc.vector.tensor_scalar_mul(out=l10f, in0=l10f,
                                        scalar1=float(TANH_CLIP))
            eall = fpool.tile([128, BPC, NCH], F32, tag="eall")
            nc.scalar.activation(out=eall.rearrange("p b c -> p (b c)"),
                                 in_=l10f,
                                 func=mybir.ActivationFunctionType.Exp)
            kmview = _view(kmT, [[1, BPC], [BPC, NCH]])
            nc.vector.tensor_mul(out=eall, in0=eall, in1=kmview)
            sums = fpool.tile([128, BPC], F32R, tag="sums")
            with nc.allow_low_precision(reason="f32r is fp32-width"):
                nc.vector.reduce_sum(out=sums, in_=eall,
                                     axis=mybir.AxisListType.X)
            dap = ps_sm.tile([1, BPC], F32, tag="sm")
            nc.tensor.matmul(dap, ones, sums, start=True, stop=True)
            lseS = fpool.tile([1, BPC], F32R, tag="lseS")
            nc.scalar.activation(out=lseS, in_=dap,
                                 func=mybir.ActivationFunctionType.Ln)
            lbp = ps_sm.tile([128, BPC], F32, tag="sm")
            nc.tensor.matmul(lbp, ones_row, lseS, start=True, stop=True)
            logp = fpool.tile([128, BPC, NCH], F32, tag="logp")
            nc.vector.tensor_sub(
                out=logp.rearrange("p b c -> p (b c)"),
                in0=l10f,
                in1=_view(bass.AP(tensor=lbp.tensor, offset=lbp.offset,
                                  ap=list(lbp.ap)),
                          [[1, BPC], [0, NCH]]))
            maskview = _view(maskT, [[1, BPC], [BPC, NCH]])
            nc.vector.copy_predicated(
                out=logp, mask=maskview,
                data=bass.AP(tensor=ninf.tensor, offset=ninf.offset,
                             ap=[ninf.ap[0], [0, BPC], [0, NCH]]))
            nc.sync.dma_start(
                out=out_d.rearrange("b (c p) -> p b c", p=128),
                in_=logp)

    nc.compile()
    return nc


def _prep_host(embeddings, step_context, mask, W_node, W_fixed, W_step, W_out):
    """Host-side marshaling: shard, transpose/pad weights, build constants."""
    import ml_dtypes
    emb = np.ascontiguousarray(
        np.asarray(embeddings, dtype=np.float32).astype(ml_dtypes.bfloat16))
    sc = np.asarray(step_context, dtype=np.float32).reshape(B, 2 * D + 1)
    msk = np.asarray(mask).reshape(B, N).astype(bool)
    km = (~msk).astype(np.float32)

    wn = np.asarray(W_node, dtype=np.float32)
    wn_in = np.ascontiguousarray(wn.reshape(2, 128, 3 * D).transpose(1, 0, 2))
    wnT = np.ascontiguousarray(wn.T)  # [768, 256]
    wnT_in = np.ascontiguousarray(wnT.reshape(6, 128, D).transpose(1, 0, 2))
    wfx = np.asarray(W_fixed, dtype=np.float32) * (1.0 / N)  # fold mean 1/N
    wfx_in = np.ascontiguousarray(wfx.reshape(2, 128, D).transpose(1, 0, 2))
    wsp = np.zeros((640, D), np.float32)
    wsp[:2 * D + 1] = np.asarray(W_step, dtype=np.float32)
    wsp_in = np.ascontiguousarray(wsp.reshape(5, 128, D).transpose(1, 0, 2))
    wout_in = np.ascontiguousarray(
        np.asarray(W_out, dtype=np.float32).reshape(2, 128, D).transpose(1, 0, 2))

    j = np.arange(D)
    hmask = np.zeros((D, H), np.float32)
    hmask[j, j // DK] = INV_SQRT_DK
    hm_in = np.ascontiguousarray(hmask.reshape(2, 128, H).transpose(1, 0, 2))

    ident = np.eye(128, dtype=np.float32)

    in_maps = []
    for k in range(NCORES):
        bs = slice(k * BPC, (k + 1) * BPC)
        scp = np.zeros((640, BPC), np.float32)
        scp[:2 * D + 1] = sc[bs].T
        scT_in = np.ascontiguousarray(
            scp.reshape(5, 128, BPC).transpose(1, 0, 2))
        kmT_in = np.ascontiguousarray(
            km[bs].reshape(BPC, NCH, 128).transpose(2, 1, 0).astype(
                ml_dtypes.bfloat16))
        mT_in = np.ascontiguousarray(
            msk[bs].reshape(BPC, NCH, 128).transpose(2, 1, 0).astype(np.uint8))
        in_maps.append({
            "emb": np.ascontiguousarray(emb[bs]),
            "kmT": kmT_in, "mT": mT_in,
            "scT": scT_in,
            "wgvb": np.ascontiguousarray(wn_in[:, :, D:2 * D]).astype(
                ml_dtypes.bfloat16),
            "wnT": wnT_in, "wfx": wfx_in, "wsp": wsp_in,
            "wout": wout_in, "hm": hm_in, "ident": ident, "identf": ident,
        })
    return in_maps


def kernel(embeddings, step_context, mask, W_node, W_fixed, W_step, W_out,
           _want_trace=False):
    if "nc" not in _CACHE:
        _CACHE["nc"] = build_program()
    nc = _CACHE["nc"]
    in_maps = _prep_host(embeddings, step_context, mask,
                         W_node, W_fixed, W_step, W_out)
    res = bass_utils.run_bass_kernel_spmd(
        nc, in_maps, core_ids=list(range(NCORES)), trace=_want_trace)
    _CACHE["last_res"] = res
    outs = [res.results[k]["out"] for k in range(NCORES)]
    full = np.concatenate(outs, axis=0).reshape(B, 1, N)
    return full.astype(np.float32)


# revision 30
# speedup vs baseline: 1.0394x; 1.0394x over previous
"""Trainium2 Bass kernel for nn_AttentionModel_47648367182457.

Pointer-network attention (Kool et al. style): one query per batch attends over
N=1024 node embeddings. Instead of materializing the O(B*N*D*3D) projection
kvl = E @ W_node (103 GFLOP), the query/glimpse vectors are back-projected
through the weights so every pass over E is a thin matmul:

  compat[b,h,n] = E[b,n,:] . qt[b,h,:]   qt = W_gk-block @ query    (contract d)
  wsum[b,h,:]  = sum_n attn[b,h,n] E[b,n,:]                         (contract n)
  logits[b,n]  = E[b,n,:] . gt[b,:]      gt = W_lk^T @ glimpse      (contract d)

Total ~2.3 GFLOP + one read of E (256 MB). Data-parallel over batch: 8 cores x
32 batches. E is transposed on-chip (PE transpose) because the d-contractions
need d on partitions; all matmuls run in float32r (~1.5e-4 rel err on HW at
1 cyc/col streaming).
"""
import sys

sys.path.insert(0, "/opt/trn_rl_repo")

import numpy as np

import concourse.bass as bass
import concourse.bacc as bacc
import concourse.tile as tile
from concourse import mybir
from concourse import bass_utils

F32 = mybir.dt.float32
F32R = mybir.dt.float32r
BF16 = mybir.dt.bfloat16

B, N, D, H = 256, 1024, 256, 8
DK = D // H              # 32
NCORES = 8
BPC = B // NCORES        # 32 batches per core
NCH = N // 128           # 8 n-chunks of 128
TANH_CLIP = 10.0
INV_SQRT_DK = 1.0 / np.sqrt(DK)
INV_SQRT_D = 1.0 / np.sqrt(D)

_CACHE = {}


def _view(ap, free_dims):
    """AP with same tensor/partition dim but custom free [step, count] list."""
    return bass.AP(tensor=ap.tensor, offset=ap.offset,
                   ap=[ap.ap[0]] + [list(e) for e in free_dims])


def _bcast_free(ap, n_rep):
    return bass.AP(tensor=ap.tensor, offset=ap.offset,
                   ap=list(ap.ap) + [[0, n_rep]])


def build_program():
    nc = bacc.Bacc("TRN2", target_bir_lowering=False, debug=False,
                   num_devices=NCORES)

    emb = nc.dram_tensor("emb", [BPC, N, D], BF16, kind="ExternalInput").ap()
    kmT_d = nc.dram_tensor("kmT", [128, NCH, BPC], BF16, kind="ExternalInput").ap()
    mT_d = nc.dram_tensor("mT", [128, NCH, BPC], mybir.dt.uint8, kind="ExternalInput").ap()
    scT_d = nc.dram_tensor("scT", [128, 5, BPC], F32R, kind="ExternalInput").ap()
    wgvb_d = nc.dram_tensor("wgvb", [128, 2, D], BF16, kind="ExternalInput").ap()
    wnT_d = nc.dram_tensor("wnT", [128, 6, D], F32R, kind="ExternalInput").ap()
    wfx_d = nc.dram_tensor("wfx", [128, 2, D], F32R, kind="ExternalInput").ap()
    wsp_d = nc.dram_tensor("wsp", [128, 5, D], F32R, kind="ExternalInput").ap()
    wout_d = nc.dram_tensor("wout", [128, 2, D], F32R, kind="ExternalInput").ap()
    hm_d = nc.dram_tensor("hm", [128, 2, H], F32, kind="ExternalInput").ap()
    ident_d = nc.dram_tensor("ident", [128, 128], F32R, kind="ExternalInput").ap()
    identf_d = nc.dram_tensor("identf", [128, 128], F32, kind="ExternalInput").ap()
    out_d = nc.dram_tensor("out", [BPC, N], F32, kind="ExternalOutput").ap()

    with tile.TileContext(nc) as tc:
        with tc.tile_pool(name="wpool", bufs=1) as wpool, \
             tc.tile_pool(name="epool", bufs=12) as epool, \
             tc.tile_pool(name="etpool", bufs=12) as etpool, \
             tc.tile_pool(name="spool", bufs=9) as spool, \
             tc.tile_pool(name="fpool", bufs=1) as fpool, \
             tc.tile_pool(name="ps_wp", bufs=1, space="PSUM") as ps_wp, \
             tc.tile_pool(name="ps_sm", bufs=7, space="PSUM") as ps_sm:

            # ------------- setup: constants / weights (DMA-cast to f32r) ----
            def load_r(name, shape, dram_ap):
                t = wpool.tile(shape, F32R, tag=name)
                nc.sync.dma_start(out=t, in_=dram_ap)
                return t

            ident = load_r("ident", [128, 128], ident_d)
            wgvb = wpool.tile([128, 2, D], BF16, tag="wgvb")
            nc.sync.dma_start(out=wgvb, in_=wgvb_d)
            wnT = load_r("wnT", [128, 6, D], wnT_d)
            wfx = load_r("wfx", [128, 2, D], wfx_d)
            wsp = load_r("wsp", [128, 5, D], wsp_d)
            wout = load_r("wout", [128, 2, D], wout_d)
            scT = load_r("scT", [128, 5, BPC], scT_d)

            ident_f = wpool.tile([128, 128], F32, tag="ident_f")
            nc.sync.dma_start(out=ident_f, in_=identf_d)
            hm = wpool.tile([128, 2, H], F32, tag="hm")
            nc.sync.dma_start(out=hm, in_=hm_d)

            ones_f = wpool.tile([128, 1], F32, tag="ones_f")
            nc.vector.memset(ones_f, 1.0)
            ones = wpool.tile([128, 1], F32R, tag="ones")
            nc.vector.tensor_copy(out=ones, in_=ones_f)
            onesrow_f = wpool.tile([1, 128], F32, tag="onesrow_f")
            nc.vector.memset(onesrow_f, 1.0)
            ones_row = wpool.tile([1, 128], F32R, tag="ones_row")
            nc.vector.tensor_copy(out=ones_row, in_=onesrow_f)
            ninf = wpool.tile([128, 1], F32, tag="ninf")
            nc.vector.memset(ninf, float("-inf"))
            zeros = wpool.tile([128, 1], F32, tag="zeros")
            nc.vector.memset(zeros, 0.0)
            ones_b = wpool.tile([128, 1], BF16, tag="ones_b")
            nc.vector.memset(ones_b, 1.0)

            # kmT[p, c, b]: keep-multiplier with n on partitions (f32, DVE use)
            kmT = wpool.tile([128, NCH, BPC], BF16, tag="kmT")
            nc.sync.dma_start(out=kmT, in_=kmT_d)
            maskT = wpool.tile([128, NCH, BPC], mybir.dt.uint8, tag="maskT")
            nc.sync.dma_start(out=maskT, in_=mT_d)

            # sc-tilde^T: scq[j(part), jh, b]
            scq = wpool.tile([128, 2, BPC], F32R, tag="scq")
            for jh in range(2):
                sp = ps_sm.tile([128, BPC], F32, tag="sm")
                for jj in range(5):
                    nc.tensor.matmul(
                        sp, wsp[:, jj, jh * 128:(jh + 1) * 128],
                        scT[:, jj, :], start=(jj == 0), stop=(jj == 4))
                nc.vector.tensor_copy(out=scq[:, jh, :], in_=sp)

            # ---------------- per-batch pipeline (software-pipelined) ----
            raw_all = fpool.tile([128, BPC, NCH], F32, tag="raw_all")
            state = {}

            def stageA(b):
                st = {}
                Enat = epool.tile([128, NCH, D], BF16, tag="enat")
                nc.sync.dma_start(
                    out=Enat,
                    in_=emb[b].rearrange("(c p) d -> p c d", p=128))
                ET = etpool.tile([128, 2, N], BF16, tag="et")
                for dh in range(2):
                    nc.sync.dma_start(
                        out=ET[:, dh, :],
                        in_=emb[b][:, dh * 128:(dh + 1) * 128],
                        transpose=True)
                mp = ps_sm.tile([1, D], F32, tag="sm")
                for c in range(NCH):
                    nc.tensor.matmul(
                        mp, ones_b, Enat[:, c, :],
                        start=(c == 0), stop=(c == NCH - 1))
                meanS = spool.tile([1, D], F32, tag="meanS")
                nc.scalar.copy(out=meanS, in_=mp)
                mtp = ps_sm.tile([128, 2], F32, tag="sm")
                for eh in range(2):
                    nc.tensor.transpose(
                        mtp[:, eh:eh + 1],
                        meanS[:, eh * 128:(eh + 1) * 128],
                        ident_f[0:1, 0:1])
                meanET = spool.tile([128, 2], BF16, tag="meanET")
                nc.vector.tensor_copy(out=meanET, in_=mtp)
                st["Enat"], st["ET"], st["meanET"] = Enat, ET, meanET
                state[b] = st

            def stageB(b):
                st = state[b]
                Enat, ET, meanET = st["Enat"], st["ET"], st["meanET"]
                qtp = ps_sm.tile([128, 2], F32, tag="sm")
                for jh in range(2):
                    for eh in range(2):
                        nc.tensor.matmul(
                            qtp[:, jh:jh + 1],
                            wfxb[:, eh, jh * 128:(jh + 1) * 128],
                            meanET[:, eh:eh + 1],
                            start=(eh == 0), stop=(eh == 1))
                qTs = spool.tile([128, 2], F32, tag="qTs")
                nc.vector.tensor_add(out=qTs, in0=qtp, in1=scq[:, :, b])
                st["qTs"] = qTs

            def stageB1b(b):
                st = state[b]
                qTs = st["qTs"]
                Qm = spool.tile([128, 2, H], F32R, tag="Qm")
                nc.gpsimd.tensor_mul(out=Qm, in0=hm, in1=_bcast_free(qTs, H))
                qp = ps_sm.tile([128, 2, H], F32, tag="sm")
                for dh in range(2):
                    for jh in range(2):
                        nc.tensor.matmul(
                            qp[:, dh, :],
                            wnT[:, jh, dh * 128:(dh + 1) * 128],
                            Qm[:, jh, :],
                            start=(jh == 0), stop=(jh == 1))
                qtS = spool.tile([128, 2, H], BF16, tag="qtS")
                nc.vector.tensor_copy(out=qtS, in_=qp)
                st["qtS"] = qtS

            def stageB2(b):
                st = state[b]
                ET, qtS = st["ET"], st["qtS"]
                ctp = ps_sm.tile([128, NCH * H], F32, tag="sm")
                for c in range(NCH):
                    for dh in range(2):
                        nc.tensor.matmul(
                            ctp[:, c * H:(c + 1) * H],
                            ET[:, dh, c * 128:(c + 1) * 128],
                            qtS[:, dh, :],
                            start=(dh == 0), stop=(dh == 1))
                expT = spool.tile([128, NCH, H], BF16, tag="expT")
                nc.scalar.activation(
                    out=expT.rearrange("p c h -> p (c h)"), in_=ctp,
                    func=mybir.ActivationFunctionType.Exp)
                nc.gpsimd.tensor_mul(
                    out=expT, in0=expT, in1=_bcast_free(kmT[:, :, b], H))
                st["expT"] = expT

            def stageC(b):
                st = state[b]
                Enat, expT = st["Enat"], st["expT"]
                part = spool.tile([128, H], F32, tag="part")
                nc.vector.reduce_sum(
                    out=part,
                    in_=_view(expT, [[1, H], [H, NCH]]),
                    axis=mybir.AxisListType.X)
                dp = ps_sm.tile([H, 1], F32, tag="sm")
                nc.tensor.matmul(dp, part, ones_f, start=True, stop=True)
                rd = spool.tile([H, 1], F32, tag="rd")
                nc.vector.reciprocal(out=rd, in_=dp)
                wp = ps_wp.tile([H, D], F32, tag="wp")
                for c in range(NCH):
                    nc.tensor.matmul(
                        wp, expT[:, c, :], Enat[:, c, :],
                        start=(c == 0), stop=(c == NCH - 1))
                wsumS = spool.tile([H, D], BF16, tag="wsumS")
                nc.scalar.activation(
                    out=wsumS, in_=wp,
                    func=mybir.ActivationFunctionType.Copy, scale=rd)
                wtp = ps_sm.tile([128, 2 * H], BF16, tag="sm")
                for dh in range(2):
                    nc.tensor.transpose(
                        wtp[:, dh * H:(dh + 1) * H],
                        wsumS[:, dh * 128:(dh + 1) * 128],
                        ident_b[0:H, 0:H])
                wsumT = spool.tile([128, 2, H], BF16, tag="wsumT")
                nc.vector.tensor_copy(out=wsumT, in_=wtp)
                st["wsumT"] = wsumT

            def stageC2(b):
                st = state[b]
                wsumT = st["wsumT"]
                htp = ps_sm.tile([128, 2], F32, tag="sm")
                for h in range(H):
                    r0 = 32 * (h % 4)
                    col = h // 4
                    for dh in range(2):
                        nc.tensor.matmul(
                            htp[r0:r0 + 32, col:col + 1],
                            wgvb[:, dh, 32 * h: 32 * h + 32],
                            wsumT[:, dh, h:h + 1],
                            start=(dh == 0), stop=(dh == 1),
                            tile_position=(0, r0))
                htS = spool.tile([128, 2], BF16, tag="htS")
                nc.vector.tensor_copy(out=htS, in_=htp)
                st["htS"] = htS

            def stageC2b(b):
                st = state[b]
                htS = st["htS"]
                glp = ps_sm.tile([128, 2], F32, tag="sm")
                for dph in range(2):
                    for hh in range(2):
                        nc.tensor.matmul(
                            glp[:, dph:dph + 1],
                            woutb[:, hh, dph * 128:(dph + 1) * 128],
                            htS[:, hh:hh + 1],
                            start=(hh == 0), stop=(hh == 1))
                glT = spool.tile([128, 2], BF16, tag="glT")
                nc.vector.tensor_copy(out=glT, in_=glp)
                g2tp = ps_sm.tile([128, 2], F32, tag="sm")
                for dh in range(2):
                    for dph in range(2):
                        nc.tensor.matmul(
                            g2tp[:, dh:dh + 1],
                            wlkb[:, dph, dh * 128:(dh + 1) * 128],
                            glT[:, dph:dph + 1],
                            start=(dph == 0), stop=(dph == 1))
                gtT = spool.tile([128, 2], BF16, tag="gtT")
                nc.vector.tensor_scalar_mul(
                    out=gtT, in0=g2tp, scalar1=float(INV_SQRT_D))
                st["gtT"] = gtT

            def stageD(b):
                st = state.pop(b)
                ET, gtT = st["ET"], st["gtT"]
                ltp = ps_sm.tile([128, 2 * NCH], F32, tag="sm")
                for c in range(NCH):
                    for dh in range(2):
                        nc.tensor.matmul(
                            ltp[:, 2 * c:2 * c + 2],
                            ET[:, dh, c * 128:(c + 1) * 128],
                            _bcast_free(gtT[:, dh:dh + 1], 2),
                            start=(dh == 0), stop=(dh == 1))
                nc.vector.tensor_copy(out=raw_all[:, b, :],
                                      in_=_view(ltp, [[2, NCH]]))

            stages = [stageA, stageB, stageB1b, stageB2, stageC,
                      stageC2, stageC2b, stageD]
            NST = len(stages)
            for step in range(BPC + NST - 1):
                for si, fn in enumerate(stages):
                    bb = step - si
                    if 0 <= bb < BPC:
                        fn(bb)

            # ---------------- batched finale over all 32 b ----------------
            flat = raw_all.rearrange("p b c -> p (b c)")
            l10 = fpool.tile([128, BPC, NCH], F32, tag="l10")
            l10f = l10.rearrange("p b c -> p (b c)")
            nc.scalar.activation(out=l10f, in_=flat,
                                 func=mybir.ActivationFunctionType.Tanh)
            nc.vector.tensor_scalar_mul(out=l10f, in0=l10f,
                                        scalar1=float(TANH_CLIP))
            eall = fpool.tile([128, BPC, NCH], F32, tag="eall")
            nc.scalar.activation(out=eall.rearrange("p b c -> p (b c)"),
                                 in_=l10f,
                                 func=mybir.ActivationFunctionType.Exp)
            kmview = _view(kmT, [[1, BPC], [BPC, NCH]])
            nc.vector.tensor_mul(out=eall, in0=eall, in1=kmview)
            sums = fpool.tile([128, BPC], F32R, tag="sums")
            with nc.allow_low_precision(reason="f32r is fp32-width"):
                nc.vector.reduce_sum(out=sums, in_=eall,
                                     axis=mybir.AxisListType.X)
            dap = ps_sm.tile([1, BPC], F32, tag="sm")
            nc.tensor.matmul(dap, ones, sums, start=True, stop=True)
            lseS = fpool.tile([1, BPC], F32R, tag="lseS")
            nc.scalar.activation(out=lseS, in_=dap,
                                 func=mybir.ActivationFunctionType.Ln)
            lbp = ps_sm.tile([128, BPC], F32, tag="sm")
            nc.tensor.matmul(lbp, ones_row, lseS, start=True, stop=True)
            logp = fpool.tile([128, BPC, NCH], F32, tag="logp")
            nc.vector.tensor_sub(
                out=logp.rearrange("p b c -> p (b c)"),
                in0=l10f,
                in1=_view(bass.AP(tensor=lbp.tensor, offset=lbp.offset,
                                  ap=list(lbp.ap)),
                          [[1, BPC], [0, NCH]]))
            maskview = _view(maskT, [[1, BPC], [BPC, NCH]])
            nc.vector.copy_predicated(
                out=logp, mask=maskview,
                data=bass.AP(tensor=ninf.tensor, offset=ninf.offset,
                             ap=[ninf.ap[0], [0, BPC], [0, NCH]]))
            nc.sync.dma_start(
                out=out_d.rearrange("b (c p) -> p b c", p=128),
                in_=logp)

    nc.compile()
    return nc


def _prep_host(embeddings, step_context, mask, W_node, W_fixed, W_step, W_out):
    """Host-side marshaling: shard, transpose/pad weights, build constants."""
    import ml_dtypes
    emb = np.ascontiguousarray(
        np.asarray(embeddings, dtype=np.float32).astype(ml_dtypes.bfloat16))
    sc = np.asarray(step_context, dtype=np.float32).reshape(B, 2 * D + 1)
    msk = np.asarray(mask).reshape(B, N).astype(bool)
    km = (~msk).astype(np.float32)

    wn = np.asarray(W_node, dtype=np.float32)
    wn_in = np.ascontiguousarray(wn.reshape(2, 128, 3 * D).transpose(1, 0, 2))
    wnT = np.ascontiguousarray(wn.T)  # [768, 256]
    wnT_in = np.ascontiguousarray(wnT.reshape(6, 128, D).transpose(1, 0, 2))
    wfx = np.asarray(W_fixed, dtype=np.float32) * (1.0 / N)  # fold mean 1/N
    wfx_in = np.ascontiguousarray(wfx.reshape(2, 128, D).transpose(1, 0, 2))
    wsp = np.zeros((640, D), np.float32)
    wsp[:2 * D + 1] = np.asarray(W_step, dtype=np.float32)
    wsp_in = np.ascontiguousarray(wsp.reshape(5, 128, D).transpose(1, 0, 2))
    wout_in = np.ascontiguousarray(
        np.asarray(W_out, dtype=np.float32).reshape(2, 128, D).transpose(1, 0, 2))

    j = np.arange(D)
    hmask = np.zeros((D, H), np.float32)
    hmask[j, j // DK] = INV_SQRT_DK
    hm_in = np.ascontiguousarray(hmask.reshape(2, 128, H).transpose(1, 0, 2))

    ident = np.eye(128, dtype=np.float32)

    in_maps = []
    for k in range(NCORES):
        bs = slice(k * BPC, (k + 1) * BPC)
        scp = np.zeros((640, BPC), np.float32)
        scp[:2 * D + 1] = sc[bs].T
        scT_in = np.ascontiguousarray(
            scp.reshape(5, 128, BPC).transpose(1, 0, 2))
        kmT_in = np.ascontiguousarray(
            km[bs].reshape(BPC, NCH, 128).transpose(2, 1, 0).astype(
                ml_dtypes.bfloat16))
        mT_in = np.ascontiguousarray(
            msk[bs].reshape(BPC, NCH, 128).transpose(2, 1, 0).astype(np.uint8))
        in_maps.append({
            "emb": np.ascontiguousarray(emb[bs]),
            "kmT": kmT_in, "mT": mT_in,
            "scT": scT_in,
            "wgvb": np.ascontiguousarray(wn_in[:, :, D:2 * D]).astype(
                ml_dtypes.bfloat16),
            "wnT": wnT_in, "wfx": wfx_in, "wsp": wsp_in,
            "wout": wout_in, "hm": hm_in, "ident": ident, "identf": ident,
        })
    return in_maps


def kernel(embeddings, step_context, mask, W_node, W_fixed, W_step, W_out,
           _want_trace=False):
    if "nc" not in _CACHE:
        _CACHE["nc"] = build_program()
    nc = _CACHE["nc"]
    in_maps = _prep_host(embeddings, step_context, mask,
                         W_node, W_fixed, W_step, W_out)
    res = bass_utils.run_bass_kernel_spmd(
        nc, in_maps, core_ids=list(range(NCORES)), trace=_want_trace)
    _CACHE["last_res"] = res
    outs = [res.results[k]["out"] for k in range(NCORES)]
    full = np.concatenate(outs, axis=0).reshape(B, 1, N)
    return full.astype(np.float32)


# revision 31
# speedup vs baseline: 1.1538x; 1.1100x over previous
"""Trainium2 Bass kernel for nn_AttentionModel_47648367182457.

Pointer-network attention (Kool et al. style): one query per batch attends over
N=1024 node embeddings. Instead of materializing the O(B*N*D*3D) projection
kvl = E @ W_node (103 GFLOP), the query/glimpse vectors are back-projected
through the weights so every pass over E is a thin matmul:

  compat[b,h,n] = E[b,n,:] . qt[b,h,:]   qt = W_gk-block @ query    (contract d)
  wsum[b,h,:]  = sum_n attn[b,h,n] E[b,n,:]                         (contract n)
  logits[b,n]  = E[b,n,:] . gt[b,:]      gt = W_lk^T @ glimpse      (contract d)

Total ~2.3 GFLOP + one read of E (256 MB). Data-parallel over batch: 8 cores x
32 batches. E is transposed on-chip (PE transpose) because the d-contractions
need d on partitions; all matmuls run in float32r (~1.5e-4 rel err on HW at
1 cyc/col streaming).
"""
import sys

sys.path.insert(0, "/opt/trn_rl_repo")

import numpy as np

import concourse.bass as bass
import concourse.bacc as bacc
import concourse.tile as tile
from concourse import mybir
from concourse import bass_utils

F32 = mybir.dt.float32
F32R = mybir.dt.float32r
BF16 = mybir.dt.bfloat16

B, N, D, H = 256, 1024, 256, 8
DK = D // H              # 32
NCORES = 8
BPC = B // NCORES        # 32 batches per core
NCH = N // 128           # 8 n-chunks of 128
TANH_CLIP = 10.0
INV_SQRT_DK = 1.0 / np.sqrt(DK)
INV_SQRT_D = 1.0 / np.sqrt(D)

_CACHE = {}


def _view(ap, free_dims):
    """AP with same tensor/partition dim but custom free [step, count] list."""
    return bass.AP(tensor=ap.tensor, offset=ap.offset,
                   ap=[ap.ap[0]] + [list(e) for e in free_dims])


def _bcast_free(ap, n_rep):
    return bass.AP(tensor=ap.tensor, offset=ap.offset,
                   ap=list(ap.ap) + [[0, n_rep]])


def build_program():
    nc = bacc.Bacc("TRN2", target_bir_lowering=False, debug=False,
                   num_devices=NCORES)

    emb = nc.dram_tensor("emb", [BPC, N, D], BF16, kind="ExternalInput").ap()
    kmT_d = nc.dram_tensor("kmT", [128, NCH, BPC], BF16, kind="ExternalInput").ap()
    mT_d = nc.dram_tensor("mT", [128, NCH, BPC], mybir.dt.uint8, kind="ExternalInput").ap()
    scT_d = nc.dram_tensor("scT", [128, 5, BPC], F32R, kind="ExternalInput").ap()
    wgvb_d = nc.dram_tensor("wgvb", [128, 2, D], BF16, kind="ExternalInput").ap()
    wnT_d = nc.dram_tensor("wnT", [128, 6, D], F32R, kind="ExternalInput").ap()
    wfx_d = nc.dram_tensor("wfx", [128, 2, D], F32R, kind="ExternalInput").ap()
    wsp_d = nc.dram_tensor("wsp", [128, 5, D], F32R, kind="ExternalInput").ap()
    wout_d = nc.dram_tensor("wout", [128, 2, D], F32R, kind="ExternalInput").ap()
    hm_d = nc.dram_tensor("hm", [128, 2, H], F32, kind="ExternalInput").ap()
    ident_d = nc.dram_tensor("ident", [128, 128], F32R, kind="ExternalInput").ap()
    identf_d = nc.dram_tensor("identf", [128, 128], F32, kind="ExternalInput").ap()
    out_d = nc.dram_tensor("out", [BPC, N], F32, kind="ExternalOutput").ap()

    with tile.TileContext(nc) as tc:
        with tc.tile_pool(name="wpool", bufs=1) as wpool, \
             tc.tile_pool(name="epool", bufs=12) as epool, \
             tc.tile_pool(name="etpool", bufs=12) as etpool, \
             tc.tile_pool(name="spool", bufs=9) as spool, \
             tc.tile_pool(name="fpool", bufs=1) as fpool, \
             tc.tile_pool(name="ps_wp", bufs=1, space="PSUM") as ps_wp, \
             tc.tile_pool(name="ps_sm", bufs=7, space="PSUM") as ps_sm:

            # ------------- setup: constants / weights (DMA-cast to f32r) ----
            def load_r(name, shape, dram_ap):
                t = wpool.tile(shape, F32R, tag=name)
                nc.sync.dma_start(out=t, in_=dram_ap)
                return t

            ident = load_r("ident", [128, 128], ident_d)
            wgvb = wpool.tile([128, 2, D], BF16, tag="wgvb")
            nc.sync.dma_start(out=wgvb, in_=wgvb_d)
            wnT = load_r("wnT", [128, 6, D], wnT_d)
            wfx = load_r("wfx", [128, 2, D], wfx_d)
            wsp = load_r("wsp", [128, 5, D], wsp_d)
            wout = load_r("wout", [128, 2, D], wout_d)
            scT = load_r("scT", [128, 5, BPC], scT_d)

            ident_f = wpool.tile([128, 128], F32, tag="ident_f")
            nc.sync.dma_start(out=ident_f, in_=identf_d)
            hm = wpool.tile([128, 2, H], F32, tag="hm")
            nc.sync.dma_start(out=hm, in_=hm_d)

            ones_f = wpool.tile([128, 1], F32, tag="ones_f")
            nc.vector.memset(ones_f, 1.0)
            ones = wpool.tile([128, 1], F32R, tag="ones")
            nc.vector.tensor_copy(out=ones, in_=ones_f)
            onesrow_f = wpool.tile([1, 128], F32, tag="onesrow_f")
            nc.vector.memset(onesrow_f, 1.0)
            ones_row = wpool.tile([1, 128], F32R, tag="ones_row")
            nc.vector.tensor_copy(out=ones_row, in_=onesrow_f)
            ninf = wpool.tile([128, 1], F32, tag="ninf")
            nc.vector.memset(ninf, float("-inf"))
            zeros = wpool.tile([128, 1], F32, tag="zeros")
            nc.vector.memset(zeros, 0.0)
            ones_b = wpool.tile([128, 1], BF16, tag="ones_b")
            nc.vector.memset(ones_b, 1.0)

            # kmT[p, c, b]: keep-multiplier with n on partitions (f32, DVE use)
            kmT = wpool.tile([128, NCH, BPC], BF16, tag="kmT")
            nc.sync.dma_start(out=kmT, in_=kmT_d)
            maskT = wpool.tile([128, NCH, BPC], mybir.dt.uint8, tag="maskT")
            nc.sync.dma_start(out=maskT, in_=mT_d)

            # sc-tilde^T: scq[j(part), jh, b]
            scq = wpool.tile([128, 2, BPC], F32R, tag="scq")
            for jh in range(2):
                sp = ps_sm.tile([128, BPC], F32, tag="sm")
                for jj in range(5):
                    nc.tensor.matmul(
                        sp, wsp[:, jj, jh * 128:(jh + 1) * 128],
                        scT[:, jj, :], start=(jj == 0), stop=(jj == 4))
                nc.vector.tensor_copy(out=scq[:, jh, :], in_=sp)

            # ---------------- per-batch pipeline (software-pipelined) ----
            raw_all = fpool.tile([128, BPC, NCH], F32, tag="raw_all")
            state = {}

            def stageA(b):
                st = {}
                Enat = epool.tile([128, NCH, D], BF16, tag="enat")
                nc.sync.dma_start(
                    out=Enat,
                    in_=emb[b].rearrange("(p i) d -> p i d", p=128))
                ET = etpool.tile([128, 2, N], BF16, tag="et")
                for dh in range(2):
                    nc.sync.dma_start(
                        out=ET[:, dh, :],
                        in_=emb[b][:, dh * 128:(dh + 1) * 128],
                        transpose=True)
                mp = ps_sm.tile([1, D], F32, tag="sm")
                for c in range(NCH):
                    nc.tensor.matmul(
                        mp, ones_b, Enat[:, c, :],
                        start=(c == 0), stop=(c == NCH - 1))
                meanS = spool.tile([1, D], F32, tag="meanS")
                nc.scalar.copy(out=meanS, in_=mp)
                mtp = ps_sm.tile([128, 2], F32, tag="sm")
                for eh in range(2):
                    nc.tensor.transpose(
                        mtp[:, eh:eh + 1],
                        meanS[:, eh * 128:(eh + 1) * 128],
                        ident_f[0:1, 0:1])
                meanET = spool.tile([128, 2], BF16, tag="meanET")
                nc.vector.tensor_copy(out=meanET, in_=mtp)
                st["Enat"], st["ET"], st["meanET"] = Enat, ET, meanET
                state[b] = st

            def stageB(b):
                st = state[b]
                Enat, ET, meanET = st["Enat"], st["ET"], st["meanET"]
                qtp = ps_sm.tile([128, 2], F32, tag="sm")
                for jh in range(2):
                    for eh in range(2):
                        nc.tensor.matmul(
                            qtp[:, jh:jh + 1],
                            wfxb[:, eh, jh * 128:(jh + 1) * 128],
                            meanET[:, eh:eh + 1],
                            start=(eh == 0), stop=(eh == 1))
                qTs = spool.tile([128, 2], F32, tag="qTs")
                nc.vector.tensor_add(out=qTs, in0=qtp, in1=scq[:, :, b])
                st["qTs"] = qTs

            def stageB1b(b):
                st = state[b]
                qTs = st["qTs"]
                Qm = spool.tile([128, 2, H], F32R, tag="Qm")
                nc.gpsimd.tensor_mul(out=Qm, in0=hm, in1=_bcast_free(qTs, H))
                qp = ps_sm.tile([128, 2, H], F32, tag="sm")
                for dh in range(2):
                    for jh in range(2):
                        nc.tensor.matmul(
                            qp[:, dh, :],
                            wnT[:, jh, dh * 128:(dh + 1) * 128],
                            Qm[:, jh, :],
                            start=(jh == 0), stop=(jh == 1))
                qtS = spool.tile([128, 2, H], BF16, tag="qtS")
                nc.vector.tensor_copy(out=qtS, in_=qp)
                st["qtS"] = qtS

            def stageB2(b):
                st = state[b]
                ET, qtS = st["ET"], st["qtS"]
                ctp = ps_sm.tile([128, NCH * H], F32, tag="sm")
                for c in range(NCH):
                    for dh in range(2):
                        nc.tensor.matmul(
                            ctp[:, c * H:(c + 1) * H],
                            ET[:, dh, c * 128:(c + 1) * 128],
                            qtS[:, dh, :],
                            start=(dh == 0), stop=(dh == 1))
                expT = spool.tile([128, NCH, H], BF16, tag="expT")
                nc.scalar.activation(
                    out=expT.rearrange("p c h -> p (c h)"), in_=ctp,
                    func=mybir.ActivationFunctionType.Exp)
                nc.gpsimd.tensor_mul(
                    out=expT, in0=expT, in1=_bcast_free(kmT[:, :, b], H))
                st["expT"] = expT

            def stageC(b):
                st = state[b]
                Enat, expT = st["Enat"], st["expT"]
                part = spool.tile([128, H], F32, tag="part")
                nc.vector.reduce_sum(
                    out=part,
                    in_=_view(expT, [[1, H], [H, NCH]]),
                    axis=mybir.AxisListType.X)
                dp = ps_sm.tile([H, 1], F32, tag="sm")
                nc.tensor.matmul(dp, part, ones_f, start=True, stop=True)
                rd = spool.tile([H, 1], F32, tag="rd")
                nc.vector.reciprocal(out=rd, in_=dp)
                wp = ps_wp.tile([H, D], F32, tag="wp")
                for c in range(NCH):
                    nc.tensor.matmul(
                        wp, expT[:, c, :], Enat[:, c, :],
                        start=(c == 0), stop=(c == NCH - 1))
                wsumS = spool.tile([H, D], BF16, tag="wsumS")
                nc.scalar.activation(
                    out=wsumS, in_=wp,
                    func=mybir.ActivationFunctionType.Copy, scale=rd)
                wtp = ps_sm.tile([128, 2 * H], BF16, tag="sm")
                for dh in range(2):
                    nc.tensor.transpose(
                        wtp[:, dh * H:(dh + 1) * H],
                        wsumS[:, dh * 128:(dh + 1) * 128],
                        ident_b[0:H, 0:H])
                wsumT = spool.tile([128, 2, H], BF16, tag="wsumT")
                nc.vector.tensor_copy(out=wsumT, in_=wtp)
                st["wsumT"] = wsumT

            def stageC2(b):
                st = state[b]
                wsumT = st["wsumT"]
                htp = ps_sm.tile([128, 2], F32, tag="sm")
                for h in range(H):
                    r0 = 32 * (h % 4)
                    col = h // 4
                    for dh in range(2):
                        nc.tensor.matmul(
                            htp[r0:r0 + 32, col:col + 1],
                            wgvb[:, dh, 32 * h: 32 * h + 32],
                            wsumT[:, dh, h:h + 1],
                            start=(dh == 0), stop=(dh == 1),
                            tile_position=(0, r0))
                htS = spool.tile([128, 2], BF16, tag="htS")
                nc.vector.tensor_copy(out=htS, in_=htp)
                st["htS"] = htS

            def stageC2b(b):
                st = state[b]
                htS = st["htS"]
                glp = ps_sm.tile([128, 2], F32, tag="sm")
                for dph in range(2):
                    for hh in range(2):
                        nc.tensor.matmul(
                            glp[:, dph:dph + 1],
                            woutb[:, hh, dph * 128:(dph + 1) * 128],
                            htS[:, hh:hh + 1],
                            start=(hh == 0), stop=(hh == 1))
                glT = spool.tile([128, 2], BF16, tag="glT")
                nc.vector.tensor_copy(out=glT, in_=glp)
                g2tp = ps_sm.tile([128, 2], F32, tag="sm")
                for dh in range(2):
                    for dph in range(2):
                        nc.tensor.matmul(
                            g2tp[:, dh:dh + 1],
                            wlkb[:, dph, dh * 128:(dh + 1) * 128],
                            glT[:, dph:dph + 1],
                            start=(dph == 0), stop=(dph == 1))
                gtT = spool.tile([128, 2], BF16, tag="gtT")
                nc.vector.tensor_scalar_mul(
                    out=gtT, in0=g2tp, scalar1=float(INV_SQRT_D))
                st["gtT"] = gtT

            def stageD(b):
                st = state.pop(b)
                ET, gtT = st["ET"], st["gtT"]
                ltp = ps_sm.tile([128, 2 * NCH], F32, tag="sm")
                for c in range(NCH):
                    for dh in range(2):
                        nc.tensor.matmul(
                            ltp[:, 2 * c:2 * c + 2],
                            ET[:, dh, c * 128:(c + 1) * 128],
                            _bcast_free(gtT[:, dh:dh + 1], 2),
                            start=(dh == 0), stop=(dh == 1))
                nc.vector.tensor_copy(out=raw_all[:, b, :],
                                      in_=_view(ltp, [[2, NCH]]))

            stages = [stageA, stageB, stageB1b, stageB2, stageC,
                      stageC2, stageC2b, stageD]
            NST = len(stages)
            for step in range(BPC + NST - 1):
                for si, fn in enumerate(stages):
                    bb = step - si
                    if 0 <= bb < BPC:
                        fn(bb)

            # ---------------- batched finale over all 32 b ----------------
            flat = raw_all.rearrange("p b c -> p (b c)")
            l10 = fpool.tile([128, BPC, NCH], F32, tag="l10")
            l10f = l10.rearrange("p b c -> p (b c)")
            nc.scalar.activation(out=l10f, in_=flat,
                                 func=mybir.ActivationFunctionType.Tanh)
            nc.vector.tensor_scalar_mul(out=l10f, in0=l10f,
                                        scalar1=float(TANH_CLIP))
            eall = fpool.tile([128, BPC, NCH], F32, tag="eall")
            nc.scalar.activation(out=eall.rearrange("p b c -> p (b c)"),
                                 in_=l10f,
                                 func=mybir.ActivationFunctionType.Exp)
            kmview = _view(kmT, [[1, BPC], [BPC, NCH]])
            nc.vector.tensor_mul(out=eall, in0=eall, in1=kmview)
            sums = fpool.tile([128, BPC], F32R, tag="sums")
            with nc.allow_low_precision(reason="f32r is fp32-width"):
                nc.vector.reduce_sum(out=sums, in_=eall,
                                     axis=mybir.AxisListType.X)
            dap = ps_sm.tile([1, BPC], F32, tag="sm")
            nc.tensor.matmul(dap, ones, sums, start=True, stop=True)
            lseS = fpool.tile([1, BPC], F32R, tag="lseS")
            nc.scalar.activation(out=lseS, in_=dap,
                                 func=mybir.ActivationFunctionType.Ln)
            lbp = ps_sm.tile([128, BPC], F32, tag="sm")
            nc.tensor.matmul(lbp, ones_row, lseS, start=True, stop=True)
            logp = fpool.tile([128, BPC, NCH], F32, tag="logp")
            nc.vector.tensor_sub(
                out=logp.rearrange("p b c -> p (b c)"),
                in0=l10f,
                in1=_view(bass.AP(tensor=lbp.tensor, offset=lbp.offset,
                                  ap=list(lbp.ap)),
                          [[1, BPC], [0, NCH]]))
            maskview = _view(maskT, [[1, BPC], [BPC, NCH]])
            nc.vector.copy_predicated(
                out=logp, mask=maskview,
                data=bass.AP(tensor=ninf.tensor, offset=ninf.offset,
                             ap=[ninf.ap[0], [0, BPC], [0, NCH]]))
            nc.sync.dma_start(
                out=out_d.rearrange("b (p c) -> p b c", p=128),
                in_=logp)

    nc.compile()
    return nc


def _prep_host(embeddings, step_context, mask, W_node, W_fixed, W_step, W_out):
    """Host-side marshaling: shard, transpose/pad weights, build constants."""
    import ml_dtypes
    emb = np.ascontiguousarray(
        np.asarray(embeddings, dtype=np.float32).astype(ml_dtypes.bfloat16))
    sc = np.asarray(step_context, dtype=np.float32).reshape(B, 2 * D + 1)
    msk = np.asarray(mask).reshape(B, N).astype(bool)
    km = (~msk).astype(np.float32)

    wn = np.asarray(W_node, dtype=np.float32)
    wn_in = np.ascontiguousarray(wn.reshape(2, 128, 3 * D).transpose(1, 0, 2))
    wnT = np.ascontiguousarray(wn.T)  # [768, 256]
    wnT_in = np.ascontiguousarray(wnT.reshape(6, 128, D).transpose(1, 0, 2))
    wfx = np.asarray(W_fixed, dtype=np.float32) * (1.0 / N)  # fold mean 1/N
    wfx_in = np.ascontiguousarray(wfx.reshape(2, 128, D).transpose(1, 0, 2))
    wsp = np.zeros((640, D), np.float32)
    wsp[:2 * D + 1] = np.asarray(W_step, dtype=np.float32)
    wsp_in = np.ascontiguousarray(wsp.reshape(5, 128, D).transpose(1, 0, 2))
    wout_in = np.ascontiguousarray(
        np.asarray(W_out, dtype=np.float32).reshape(2, 128, D).transpose(1, 0, 2))

    j = np.arange(D)
    hmask = np.zeros((D, H), np.float32)
    hmask[j, j // DK] = INV_SQRT_DK
    hm_in = np.ascontiguousarray(hmask.reshape(2, 128, H).transpose(1, 0, 2))

    ident = np.eye(128, dtype=np.float32)

    in_maps = []
    for k in range(NCORES):
        bs = slice(k * BPC, (k + 1) * BPC)
        scp = np.zeros((640, BPC), np.float32)
        scp[:2 * D + 1] = sc[bs].T
        scT_in = np.ascontiguousarray(
            scp.reshape(5, 128, BPC).transpose(1, 0, 2))
        kmT_in = np.ascontiguousarray(
            km[bs].reshape(BPC, 128, NCH).transpose(1, 2, 0).astype(
                ml_dtypes.bfloat16))
        mT_in = np.ascontiguousarray(
            msk[bs].reshape(BPC, 128, NCH).transpose(1, 2, 0).astype(np.uint8))
        in_maps.append({
            "emb": np.ascontiguousarray(emb[bs]),
            "kmT": kmT_in, "mT": mT_in,
            "scT": scT_in,
            "wgvb": np.ascontiguousarray(wn_in[:, :, D:2 * D]).astype(
                ml_dtypes.bfloat16),
            "wnT": wnT_in, "wfx": wfx_in, "wsp": wsp_in,
            "wout": wout_in, "hm": hm_in, "ident": ident, "identf": ident,
        })
    return in_maps


def kernel(embeddings, step_context, mask, W_node, W_fixed, W_step, W_out,
           _want_trace=False):
    if "nc" not in _CACHE:
        _CACHE["nc"] = build_program()
    nc = _CACHE["nc"]
    in_maps = _prep_host(embeddings, step_context, mask,
                         W_node, W_fixed, W_step, W_out)
    res = bass_utils.run_bass_kernel_spmd(
        nc, in_maps, core_ids=list(range(NCORES)), trace=_want_trace)
    _CACHE["last_res"] = res
    outs = [res.results[k]["out"] for k in range(NCORES)]
    full = np.concatenate(outs, axis=0).reshape(B, 1, N)
    return full.astype(np.float32)
